# revision 3
# baseline (speedup 1.0000x reference)
"""KnnLoss Trainium2 kernel.

Problem: B=2, N=8192 points in [0,1)^3, mask (B,N,16). For each point, find
its 8 nearest neighbors (squared L2, via s = 2*q.c - |c|^2 which is a
per-row constant shift of -d2), replace out-of-radius neighbors with the
nearest (self) index, gather mask rows at the neighbor indices, and
accumulate sum_s |mask[n,s] - mask[nn,s]|. Final loss = total / (B*N*k).

Sharding: 8 cores, each handles one (batch, query-block) pair: core c ->
batch c//4, queries (c%4)*2048 .. +2048. Candidates/gather table are the
full per-batch pc/mask, fed per-core as SPMD data.

Per core pipeline (per 128-query tile):
  PE:  16 matmuls [4,128]x[4,512] -> PSUM s-chunks
  ACT: copy PSUM -> SBUF row [128, 8192]
  DVE: max8 + find_index8 -> top-8 values/indices; radius filter; index fixup
  SWDGE: indirect gather of mask rows [128, 8x16]
  DVE+ACT: |diff| + accumulate -> per-query partial loss
"""

import numpy as np

import concourse.bass as bass
import concourse.mybir as mybir
import concourse.tile as tile
from concourse import bacc
from concourse.bass import IndirectOffsetOnAxis, ts
from concourse.bass_utils import run_bass_kernel_spmd

B = 2
N = 8192
KS = 16
KNN = 8
R2 = np.float32(0.1) * np.float32(0.1)  # 0.01 squared radius
NCORES = 8
QPC = B * N // NCORES  # 2048 queries per core
NT = QPC // 128        # 16 query tiles per core
CH = 512               # candidate chunk (one PSUM bank)
NCH = N // CH          # 16 chunks

F32 = mybir.dt.float32
U32 = mybir.dt.uint32

_CACHE = {}


def _body(tc, pc_all, pc_q, mask_g, mask_q, loss_out):
    nc = tc.nc
    import contextlib
    with contextlib.ExitStack() as ctx:
        cpool = ctx.enter_context(tc.tile_pool(name="const", bufs=1))
        rpool = ctx.enter_context(tc.tile_pool(name="rows", bufs=2))
        spool = ctx.enter_context(tc.tile_pool(name="small", bufs=3))
        ppool = ctx.enter_context(tc.tile_pool(name="psum", bufs=6, space="PSUM"))

        # ---- setup: candidate matrix Cp = [x; y; z; -|c|^2], query matrix
        # Qs = [2x; 2y; 2z; 1] so that s = Qs[:,q].T @ Cp[:,c] = 2 q.c - |c|^2
        Cp = cpool.tile([4, N], F32)
        Qs = cpool.tile([4, QPC], F32)
        # memset the whole tile to 1.0 so row 3 (the "ones" row) is ready,
        # then overwrite rows 0-2 with the coords (DVE/ACT can't start at
        # partition 3, so row 3 is never touched directly by compute).
        nc.vector.memset(Qs[0:4, :], 1.0)
        nc.sync.dma_start(out=Qs[0:3, :], in_=pc_q.ap().rearrange("n d -> d n"))
        nc.scalar.mul(Qs[0:3, :], Qs[0:3, :], 2.0)
        nc.sync.dma_start(out=Cp[0:3, :], in_=pc_all.ap().rearrange("n d -> d n"))

        sq3 = cpool.tile([3, N], F32)
        nc.vector.tensor_mul(sq3[:, :], Cp[0:3, :], Cp[0:3, :])
        nones3 = cpool.tile([3, 1], F32)
        nc.vector.memset(nones3[:, :], -1.0)
        csqrow = cpool.tile([1, N], F32)
        for ch in range(NCH):
            pcsq = ppool.tile([128, CH], F32, tag="ps")
            nc.tensor.matmul(
                out=pcsq[0:1, :],
                lhsT=nones3[:, :],
                rhs=sq3[:, ts(ch, CH)],
                start=True,
                stop=True,
            )
            nc.scalar.copy(csqrow[0:1, ts(ch, CH)], pcsq[0:1, :])
        # row 3 of Cp = -|c|^2 (DMA has no partition-start restriction)
        nc.sync.dma_start(out=Cp[3:4, :], in_=csqrow[0:1, :])

        # ---- main loop over query tiles
        for t in range(NT):
            nrow = rpool.tile([128, N], F32)
            for ch in range(NCH):
                ps = ppool.tile([128, CH], F32, tag="ps")
                nc.tensor.matmul(
                    out=ps[:, :],
                    lhsT=Qs[:, ts(t, 128)],
                    rhs=Cp[:, ts(ch, CH)],
                    start=True,
                    stop=True,
                )
                nc.scalar.copy(nrow[:, ts(ch, CH)], ps[:, :])

            # top-8 values (descending) and their indices
            tv = spool.tile([128, 8], F32)
            nc.vector.max(out=tv[:, :], in_=nrow[:, :])
            ti = spool.tile([128, 8], U32)
            nc.vector.max_index(out=ti[:, :], in_max=tv[:, :], in_values=nrow[:, :])

            # keep_j = (s_j >= s_0 - R2)  <=>  d2_j <= R2
            th = spool.tile([128, 1], F32)
            nc.vector.tensor_scalar(
                out=th[:, :], in0=tv[:, 0:1], scalar1=-float(R2), scalar2=None,
                op0=mybir.AluOpType.add,
            )
            kp = spool.tile([128, 8], F32)
            nc.vector.tensor_scalar(
                out=kp[:, :], in0=tv[:, :], scalar1=th[:, :], scalar2=None,
                op0=mybir.AluOpType.is_ge,
            )

            # idx_fixed = idx0 + keep * (idx - idx0)   (all exact in f32)
            idxf = spool.tile([128, 8], F32)
            nc.vector.tensor_copy(idxf[:, :], ti[:, :])
            self_bc = idxf[:, 0:1].to_broadcast([128, 8])
            dl = spool.tile([128, 8], F32)
            nc.vector.tensor_tensor(
                out=dl[:, :], in0=idxf[:, :], in1=self_bc, op=mybir.AluOpType.subtract
            )
            nc.vector.tensor_mul(dl[:, :], dl[:, :], kp[:, :])
            fi = spool.tile([128, 8], F32)
            nc.vector.tensor_tensor(
                out=fi[:, :], in0=dl[:, :], in1=self_bc, op=mybir.AluOpType.add
            )
            fio = spool.tile([128, 8], U32)
            nc.vector.tensor_copy(fio[:, :], fi[:, :])

            # gather neighbor mask rows: [128, 8, 16]
            # ([P,1]-shaped offsets per call: the multi-index offset form
            # compiles but silently transfers nothing on HW)
            gt = spool.tile([128, KNN, KS], F32)
            for j in range(KNN):
                nc.gpsimd.indirect_dma_start(
                    out=gt[:, j, :],
                    out_offset=None,
                    in_=mask_g.ap(),
                    in_offset=IndirectOffsetOnAxis(ap=fio[:, j : j + 1], axis=0),
                )

            # own mask rows for this tile
            mq = spool.tile([128, KS], F32)
            nc.sync.dma_start(out=mq[:, :], in_=mask_q.ap()[ts(t, 128), :])

            # |own - neighbor| summed over (j, s) per query
            df = spool.tile([128, KNN, KS], F32)
            mq_bc = mq[:, :].rearrange("p (o s) -> p o s", o=1).to_broadcast(
                [128, KNN, KS]
            )
            nc.vector.tensor_tensor(
                out=df[:, :, :], in0=gt[:, :, :], in1=mq_bc,
                op=mybir.AluOpType.subtract,
            )
            ab = spool.tile([128, KNN, KS], F32)
            lt = spool.tile([128, 1], F32)
            nc.scalar.activation(
                out=ab[:, :, :], in_=df[:, :, :],
                func=mybir.ActivationFunctionType.Abs,
                accum_out=lt[:, :],
            )
            nc.sync.dma_start(out=loss_out.ap()[:, t : t + 1], in_=lt[:, :])


def build_nc():
    nc = bacc.Bacc(
        "TRN2", target_bir_lowering=False, debug=False, num_devices=NCORES
    )
    pc_all = nc.dram_tensor("pc_all", [N, 3], F32, kind="ExternalInput")
    pc_q = nc.dram_tensor("pc_q", [QPC, 3], F32, kind="ExternalInput")
    mask_g = nc.dram_tensor("mask_g", [N, KS], F32, kind="ExternalInput")
    mask_q = nc.dram_tensor("mask_q", [QPC, KS], F32, kind="ExternalInput")
    loss_out = nc.dram_tensor("loss_out", [128, NT], F32, kind="ExternalOutput")
    with tile.TileContext(nc) as tc:
        _body(tc, pc_all, pc_q, mask_g, mask_q, loss_out)
    nc.compile()
    return nc


def make_in_maps(pc, mask):
    pc = np.ascontiguousarray(np.asarray(pc), dtype=np.float32)
    mask = np.ascontiguousarray(np.asarray(mask), dtype=np.float32)
    in_maps = []
    for c in range(NCORES):
        b, qb = divmod(c, NCORES // B)
        sl = slice(qb * QPC, (qb + 1) * QPC)
        in_maps.append(
            {
                "pc_all": pc[b],
                "pc_q": np.ascontiguousarray(pc[b][sl]),
                "mask_g": mask[b],
                "mask_q": np.ascontiguousarray(mask[b][sl]),
            }
        )
    return in_maps


def kernel(pc, mask):
    if "nc" not in _CACHE:
        _CACHE["nc"] = build_nc()
    nc = _CACHE["nc"]
    res = run_bass_kernel_spmd(nc, make_in_maps(pc, mask), list(range(NCORES)))
    total = 0.0
    for r in res.results:
        total += r["loss_out"].astype(np.float64).sum()
    return np.float32(total / (B * N * KNN))
